# Initial kernel scaffold
#
"""Sigmoid-attention Bass kernel for TRN2, 8 NeuronCores (batch-parallel).

Problem (per batch element b, one per core):
    S = Q^T K            [2048, 2048]   (contract over d=128)
    P = sigmoid(S/sqrt(128))
    O = V P              [128, 2048]

Layout notes:
  - d=128 maps to the SBUF partition dim for Q/K/V, so S-tiles come out of
    the PE as [n_tile=128, m] directly.
  - The O matmul contracts over n, so V must be transposed once ([v,n] ->
    [n,v] blocks); done on the PE with an identity matmul, interleaved with
    the first m-half so it hides under the ScalarE sigmoid.
  - fp32r matmuls run at 1 col/cycle (vs 4 for plain fp32) with fp32
    accumulation in PSUM.
  - ScalarE applies sigmoid with the 1/sqrt(128) scale fused, reading S
    straight from PSUM in 1024-wide calls.
  - PSUM budget (8 banks): S double-buffer 4 + O accumulator 2 + V-transpose
    staging 2.
  - O-matmuls are emitted one iteration late so the PE's in-order stream
    never waits on the sigmoid of the current iteration.
"""

import numpy as np

import concourse.bass as bass
import concourse.tile as tile
from concourse import bacc, mybir
from concourse.bass_utils import run_bass_kernel_spmd
from concourse.masks import make_identity

B, D, N = 8, 128, 2048
NT = N // 128            # 16 n-tiles of 128
MH = 2                   # m halves
MW = N // MH             # 1024 columns per half
CH = MW // 512           # 512-wide matmul chunks per half
SCALE = float(1.0 / np.sqrt(128.0))
F32 = mybir.dt.float32
F32R = mybir.dt.float32r
SIG = mybir.ActivationFunctionType.Sigmoid

_CACHED_NC = None


def build_nc():
    nc = bacc.Bacc("TRN2", target_bir_lowering=False, debug=False, num_devices=B,
                   enable_asserts=False)
    # Q/K declared float32r in DRAM: the DMA lands them in f32r SBUF tiles
    # directly (the PE rounds on use; measured accuracy matches the explicit
    # DVE-rounded path), eliminating the whole stage+cast prologue chain.
    q_ext = nc.dram_tensor("Q", [D, N], F32R, kind="ExternalInput").ap()
    k_ext = nc.dram_tensor("K", [D, N], F32R, kind="ExternalInput").ap()
    v_ext = nc.dram_tensor("V", [D, N], F32, kind="ExternalInput").ap()
    out_ext = nc.dram_tensor("out", [D, N], F32, kind="ExternalOutput").ap()

    with tile.TileContext(nc) as tc:
        with (
            tc.tile_pool(name="sb", bufs=1) as sb,
            tc.tile_pool(name="pp", bufs=4) as pp,
            tc.tile_pool(name="ob", bufs=4) as ob,
            # PSUM: S tiles double-buffered (4 banks) + two O-accumulator
            # slots (4 banks). During h0 the second O slot is idle, so the
            # V-transpose staging tiles cycle through it (same tag).
            tc.tile_pool(name="ps", bufs=2, space="PSUM") as ps,
            tc.tile_pool(name="po", bufs=2, space="PSUM") as po,
        ):
            q_r = sb.tile([D, N], F32R, tag="qr", name="q_r")
            k_r = sb.tile([D, N], F32R, tag="kr", name="k_r")
            v_sb = sb.tile([D, N], F32, tag="v", name="v_sb")
            vt_sb = sb.tile([D, N], F32R, tag="vt", name="vt_sb")

            # DMA packets are per partition-row (~158ns each regardless of
            # byte count), so narrow column chunks waste packet slots; load
            # full 1024-col blocks (4KB rows) in consumption order, spread
            # over both HWDGE queues (SP + ScalarE).
            nc.scalar.dma_start(out=q_r[:, 0:384], in_=q_ext[:, 0:384])
            nc.sync.dma_start(out=k_r[:, 0:MW], in_=k_ext[:, 0:MW])
            nc.scalar.dma_start(out=q_r[:, 384:MW], in_=q_ext[:, 384:MW])

            # Sigmoid table load (~1.3us) on the ACT stream right after its
            # first DMA issue, overlapping the transfers.
            warm = sb.tile([D, 1], F32, tag="warm", name="warm")
            nc.gpsimd.memset(warm[:], 0.0)
            warm2 = sb.tile([D, 1], F32, tag="warm2", name="warm2")
            nc.scalar.activation(warm2[:], warm[:], SIG)

            ident = sb.tile([D, D], F32, tag="ident", name="ident")
            make_identity(nc, ident[:])

            # Gapless dummy fp32 matmuls keep the PE busy through the DMA
            # wait: the HAM clock-gate sees ~3.4us of sustained activity and
            # un-throttles to 2.4GHz before the first real matmul, so the
            # early S-tile rotation never runs at the 1.2GHz mid p-state.
            junk = sb.tile([D, 512], F32, tag="junk", name="junk")
            nc.gpsimd.memset(junk[:], 0.0)
            wps = po.tile([D, 512], F32, tag="o", name="warm_ps")
            for w in range(3):
                cols = 512 if w < 2 else 256
                nc.tensor.matmul(wps[:, 0:cols], lhsT=ident[:],
                                 rhs=junk[:, 0:cols], start=True, stop=True)

            nc.sync.dma_start(out=v_sb[:, 0:512], in_=v_ext[:, 0:512])
            nc.scalar.dma_start(out=v_sb[:, 512:MW], in_=v_ext[:, 512:MW])
            nc.sync.dma_start(out=q_r[:, MW:N], in_=q_ext[:, MW:N])
            nc.scalar.dma_start(out=v_sb[:, MW:N], in_=v_ext[:, MW:N])
            nc.sync.dma_start(out=k_r[:, MW:N], in_=k_ext[:, MW:N])

            # O-matmuls are deferred two iterations behind the S-matmuls so
            # the PE's in-order stream never waits on the current sigmoid.
            pending = []

            def flush(p):
                o_ps, p_t, n, h, last = p
                for c in range(CH):
                    nc.tensor.matmul(
                        o_ps[:, bass.ts(c, 512)],
                        lhsT=vt_sb[:, bass.ts(n, D)],
                        rhs=p_t[:, bass.ts(c, 512)],
                        start=(n == 0),
                        stop=(n == NT - 1),
                    )
                    if last:
                        # Drain chunk c right after ITS final O-matmul (each
                        # 512-chunk is its own accumulation group), in
                        # 512-wide pieces (2KB rows keep DMA packets
                        # efficient). h0's drain runs mid-kernel: keep its
                        # issues OFF the ScalarE stream (an ACT-issued DMA
                        # would stall the sigmoid pipeline on the copies);
                        # only h1's drain, emitted after the last sigmoid,
                        # may use the ScalarE queue + ScalarE copy.
                        o_out = ob.tile([D, 512], F32, tag="o_out",
                                        name=f"o_out{h}_{c}")
                        final = h == MH - 1
                        if final and c % 2:
                            nc.scalar.copy(o_out[:], o_ps[:, bass.ts(c, 512)])
                        else:
                            nc.vector.tensor_copy(o_out[:], o_ps[:, bass.ts(c, 512)])
                        dma_eng = nc.scalar if (final and c % 2) else nc.sync
                        dma_eng.dma_start(
                            out=out_ext[:, h * MW + c * 512 : h * MW + (c + 1) * 512],
                            in_=o_out[:],
                        )

            def transpose_v(j):
                # Stage through the idle second O-accumulator slot (tag "o")
                # so transposes never contend with the S-tile rotation.
                tp = po.tile([D, D], F32, tag="o", name=f"tp{j}")
                nc.tensor.transpose(tp[:], v_sb[:, bass.ts(j, D)], ident[:])
                nc.vector.tensor_copy(vt_sb[:, bass.ts(j, D)], tp[:])

            for h in range(MH):
                o_ps = po.tile([D, MW], F32, tag="o", name=f"o_ps{h}")
                for n in range(NT):
                    s_ps = ps.tile([D, MW], F32, tag="s", name=f"s{h}_{n}")
                    for c in range(CH):
                        nc.tensor.matmul(
                            s_ps[:, bass.ts(c, 512)],
                            lhsT=q_r[:, bass.ts(n, D)],
                            rhs=k_r[:, h * MW + c * 512 : h * MW + (c + 1) * 512],
                            start=True,
                            stop=True,
                        )
                    # V-transposes lag one iteration behind so the PE's
                    # in-order stream never parks on the V load; transpose j
                    # is still one iteration ahead of its consumer O(j).
                    if h == 0 and n >= 1:
                        transpose_v(n - 1)
                    p_t = pp.tile([D, MW], F32R, tag="p", name=f"p{h}_{n}")
                    nc.scalar.activation(p_t[:], s_ps[:], SIG, scale=SCALE)
                    pending.append((o_ps, p_t, n, h, n == NT - 1))
                    if len(pending) > 2:
                        flush(pending.pop(0))
                if h == 0:
                    transpose_v(NT - 1)
            while pending:
                flush(pending.pop(0))

    nc.compile()
    return nc


def kernel(**inputs):
    global _CACHED_NC
    Q = np.ascontiguousarray(inputs["Q"], dtype=np.float32)
    K = np.ascontiguousarray(inputs["K"], dtype=np.float32)
    V = np.ascontiguousarray(inputs["V"], dtype=np.float32)
    assert Q.shape == (B, D, N), Q.shape

    if _CACHED_NC is None:
        _CACHED_NC = build_nc()
    nc = _CACHED_NC

    in_maps = [{"Q": Q[i], "K": K[i], "V": V[i]} for i in range(B)]
    res = run_bass_kernel_spmd(nc, in_maps, core_ids=list(range(B)))
    out = np.stack([res.results[i]["out"] for i in range(B)], axis=0)
    return out.astype(np.float32, copy=False)


if __name__ == "__main__":
    rng = np.random.default_rng(0)
    ins = {
        "Q": rng.standard_normal((B, D, N)).astype(np.float32),
        "K": rng.standard_normal((B, D, N)).astype(np.float32),
        "V": rng.standard_normal((B, D, N)).astype(np.float32),
    }
    out = kernel(**ins)
    print("kernel output", out.shape, out.dtype)



# revision 1
# speedup vs baseline: 1.1500x; 1.1500x over previous
"""Sigmoid-attention Bass kernel for TRN2, 8 NeuronCores (batch-parallel).

Problem (per batch element b, one per core):
    S = Q^T K            [2048, 2048]   (contract over d=128)
    P = sigmoid(S/sqrt(128))
    O = V P              [128, 2048]

Layout notes:
  - d=128 maps to the SBUF partition dim for Q/K/V, so S-tiles come out of
    the PE as [n_tile=128, m] directly.
  - The O matmul contracts over n, so V must be transposed once ([v,n] ->
    [n,v] blocks); done on the PE with an identity matmul, interleaved with
    the first m-half so it hides under the ScalarE sigmoid.
  - fp32r matmuls run at 1 col/cycle (vs 4 for plain fp32) with fp32
    accumulation in PSUM.
  - ScalarE applies sigmoid with the 1/sqrt(128) scale fused, reading S
    straight from PSUM in 1024-wide calls.
  - PSUM budget (8 banks): S double-buffer 4 + O accumulator 2 + V-transpose
    staging 2.
  - O-matmuls are emitted one iteration late so the PE's in-order stream
    never waits on the sigmoid of the current iteration.
"""

import numpy as np

import concourse.bass as bass
import concourse.tile as tile
from concourse import bacc, mybir
from concourse.bass_utils import run_bass_kernel_spmd
from concourse.masks import make_identity

B, D, N = 8, 128, 2048
NT = N // 128            # 16 n-tiles of 128
MH = 2                   # m halves
MW = N // MH             # 1024 columns per half
CH = MW // 512           # 512-wide matmul chunks per half
SCALE = float(1.0 / np.sqrt(128.0))
F32 = mybir.dt.float32
F32R = mybir.dt.float32r
SIG = mybir.ActivationFunctionType.Sigmoid

_CACHED_NC = None


def build_nc():
    nc = bacc.Bacc("TRN2", target_bir_lowering=False, debug=False, num_devices=B,
                   enable_asserts=False)
    # Q/K declared float32r in DRAM: the DMA lands them in f32r SBUF tiles
    # directly (the PE rounds on use; measured accuracy matches the explicit
    # DVE-rounded path), eliminating the whole stage+cast prologue chain.
    q_ext = nc.dram_tensor("Q", [D, N], F32R, kind="ExternalInput").ap()
    k_ext = nc.dram_tensor("K", [D, N], F32R, kind="ExternalInput").ap()
    v_ext = nc.dram_tensor("V", [D, N], F32, kind="ExternalInput").ap()
    out_ext = nc.dram_tensor("out", [D, N], F32, kind="ExternalOutput").ap()

    with tile.TileContext(nc) as tc:
        with (
            tc.tile_pool(name="sb", bufs=1) as sb,
            tc.tile_pool(name="pp", bufs=4) as pp,
            tc.tile_pool(name="ob", bufs=4) as ob,
            # PSUM: S tiles double-buffered (4 banks) + two O-accumulator
            # slots (4 banks). During h0 the second O slot is idle, so the
            # V-transpose staging tiles cycle through it (same tag).
            tc.tile_pool(name="ps", bufs=2, space="PSUM") as ps,
            tc.tile_pool(name="po", bufs=2, space="PSUM") as po,
        ):
            q_r = sb.tile([D, N], F32R, tag="qr", name="q_r")
            k_r = sb.tile([D, N], F32R, tag="kr", name="k_r")
            v_sb = sb.tile([D, N], F32, tag="v", name="v_sb")
            vt_sb = sb.tile([D, N], F32R, tag="vt", name="vt_sb")

            # DMA packets are per partition-row (~158ns each regardless of
            # byte count), so narrow column chunks waste packet slots; load
            # full 1024-col blocks (4KB rows) in consumption order, spread
            # over both HWDGE queues (SP + ScalarE).
            nc.scalar.dma_start(out=q_r[:, 0:384], in_=q_ext[:, 0:384])
            nc.sync.dma_start(out=k_r[:, 0:MW], in_=k_ext[:, 0:MW])
            nc.scalar.dma_start(out=q_r[:, 384:MW], in_=q_ext[:, 384:MW])

            # Sigmoid table load (~1.3us) on the ACT stream right after its
            # first DMA issue, overlapping the transfers.
            warm = sb.tile([D, 1], F32, tag="warm", name="warm")
            nc.gpsimd.memset(warm[:], 0.0)
            warm2 = sb.tile([D, 1], F32, tag="warm2", name="warm2")
            nc.scalar.activation(warm2[:], warm[:], SIG)

            ident = sb.tile([D, D], F32, tag="ident", name="ident")
            make_identity(nc, ident[:])

            # Gapless dummy fp32 matmuls keep the PE busy through the DMA
            # wait: the HAM clock-gate sees ~3.4us of sustained activity and
            # un-throttles to 2.4GHz before the first real matmul, so the
            # early S-tile rotation never runs at the 1.2GHz mid p-state.
            junk = sb.tile([D, 512], F32, tag="junk", name="junk")
            nc.gpsimd.memset(junk[:], 0.0)
            wps = po.tile([D, 512], F32, tag="o", name="warm_ps")
            for w in range(3):
                cols = 512 if w < 2 else 256
                nc.tensor.matmul(wps[:, 0:cols], lhsT=ident[:],
                                 rhs=junk[:, 0:cols], start=True, stop=True)

            nc.sync.dma_start(out=v_sb[:, 0:512], in_=v_ext[:, 0:512])
            nc.scalar.dma_start(out=v_sb[:, 512:MW], in_=v_ext[:, 512:MW])
            nc.sync.dma_start(out=q_r[:, MW:N], in_=q_ext[:, MW:N])
            nc.scalar.dma_start(out=v_sb[:, MW:N], in_=v_ext[:, MW:N])
            nc.sync.dma_start(out=k_r[:, MW:N], in_=k_ext[:, MW:N])

            # O-matmuls are deferred two iterations behind the S-matmuls so
            # the PE's in-order stream never waits on the current sigmoid.
            pending = []

            def flush(p):
                o_ps, p_t, n, h, last = p
                for c in range(CH):
                    nc.tensor.matmul(
                        o_ps[:, bass.ts(c, 512)],
                        lhsT=vt_sb[:, bass.ts(n, D)],
                        rhs=p_t[:, bass.ts(c, 512)],
                        start=(n == 0),
                        stop=(n == NT - 1),
                    )
                    if last:
                        # Drain chunk c right after ITS final O-matmul (each
                        # 512-chunk is its own accumulation group), in
                        # 512-wide pieces (2KB rows keep DMA packets
                        # efficient). h0's drain runs mid-kernel: keep its
                        # issues OFF the ScalarE stream (an ACT-issued DMA
                        # would stall the sigmoid pipeline on the copies);
                        # only h1's drain, emitted after the last sigmoid,
                        # may use the ScalarE queue + ScalarE copy.
                        o_out = ob.tile([D, 512], F32, tag="o_out",
                                        name=f"o_out{h}_{c}")
                        final = h == MH - 1
                        if final and c % 2:
                            nc.scalar.copy(o_out[:], o_ps[:, bass.ts(c, 512)])
                        else:
                            nc.vector.tensor_copy(o_out[:], o_ps[:, bass.ts(c, 512)])
                        dma_eng = nc.scalar if (final and c % 2) else nc.sync
                        dma_eng.dma_start(
                            out=out_ext[:, h * MW + c * 512 : h * MW + (c + 1) * 512],
                            in_=o_out[:],
                        )

            def transpose_v(j):
                # Stage through the idle second O-accumulator slot (tag "o")
                # so transposes never contend with the S-tile rotation.
                tp = po.tile([D, D], F32, tag="o", name=f"tp{j}")
                nc.tensor.transpose(tp[:], v_sb[:, bass.ts(j, D)], ident[:])
                nc.vector.tensor_copy(vt_sb[:, bass.ts(j, D)], tp[:])

            for h in range(MH):
                o_ps = po.tile([D, MW], F32, tag="o", name=f"o_ps{h}")
                for n in range(NT):
                    s_ps = ps.tile([D, MW], F32, tag="s", name=f"s{h}_{n}")
                    for c in range(CH):
                        nc.tensor.matmul(
                            s_ps[:, bass.ts(c, 512)],
                            lhsT=q_r[:, bass.ts(n, D)],
                            rhs=k_r[:, h * MW + c * 512 : h * MW + (c + 1) * 512],
                            start=True,
                            stop=True,
                        )
                    # V-transposes lag one iteration behind so the PE's
                    # in-order stream never parks on the V load; transpose j
                    # is still one iteration ahead of its consumer O(j).
                    if h == 0 and n >= 1:
                        transpose_v(n - 1)
                    p_t = pp.tile([D, MW], F32R, tag="p", name=f"p{h}_{n}")
                    nc.scalar.activation(p_t[:], s_ps[:], SIG, scale=SCALE)
                    pending.append((o_ps, p_t, n, h, n == NT - 1))
                    if len(pending) > 2:
                        flush(pending.pop(0))
                if h == 0:
                    transpose_v(NT - 1)
            while pending:
                flush(pending.pop(0))

    nc.compile()
    return nc


def kernel(**inputs):
    global _CACHED_NC
    Q = np.ascontiguousarray(inputs["Q"], dtype=np.float32)
    K = np.ascontiguousarray(inputs["K"], dtype=np.float32)
    V = np.ascontiguousarray(inputs["V"], dtype=np.float32)
    assert Q.shape == (B, D, N), Q.shape

    if _CACHED_NC is None:
        _CACHED_NC = build_nc()
    nc = _CACHED_NC

    in_maps = [{"Q": Q[i], "K": K[i], "V": V[i]} for i in range(B)]
    res = run_bass_kernel_spmd(nc, in_maps, core_ids=list(range(B)))
    out = np.stack([res.results[i]["out"] for i in range(B)], axis=0)
    return out.astype(np.float32, copy=False)


if __name__ == "__main__":
    rng = np.random.default_rng(0)
    ins = {
        "Q": rng.standard_normal((B, D, N)).astype(np.float32),
        "K": rng.standard_normal((B, D, N)).astype(np.float32),
        "V": rng.standard_normal((B, D, N)).astype(np.float32),
    }
    out = kernel(**ins)
    print("kernel output", out.shape, out.dtype)

